# revision 26
# baseline (speedup 1.0000x reference)
"""Trainium2 Bass kernel for nn_Attention_86646670230179 (eager MHA, f32 I/O).

Strategy (8 NeuronCores, tensor-parallel over heads, collective-free):
  - Each core owns 2 of the 16 heads (a 128-row slice of the internal dim).
  - Inputs stream in 1 MB chunks ([128, KT, 512] per (batch, n-tile, tensor))
    in consumption order on the sync queue. Score scale (1/8) folded into Wq.
  - Batch-0 projections run as a dedicated phase (DMA-bound; PSUM ping-pong,
    copy-outs on the idle ScalarE via activation(Identity, bias)). Batch-1
    projections are chopped into ~1 us closures fed one per attention step
    under need-by tags (chunk (k/v, n) is first used at step 4n of attn(1)),
    so the exp stream never pauses for a second projection phase. v projects
    directly into natural layout (lhsT = x chunk), avoiding PE transposes.
  - Attention per 512-query block: scores^T via PE row-tiled matmuls (two
    heads packed, computed one step ahead of exp so feed/drain PE work never
    blocks the exp stream), exp on ScalarE ([128,1024] tiles, no max
    subtraction: scores ~ N(0,1)), PV accumulation with an appended
    ones-column producing unnormalized outputs + row sums in one PSUM group.
  - po PSUM is copied to SBUF at block end (frees the bank); normalization is
    deferred one block: the [1,512] sum rows are DMA-shuffled (sync HW DGE;
    gpsimd software DGE measured ~10 us slower per chain) into a [128,4]
    layout so one DVE reciprocal covers both heads in 8 columns of work (DVE
    cost scales with free-dim size only), then gpsimd partition-broadcast +
    DVE multiply. Out-projection tiles drain one per late attention step
    (their ot input is ready only ~5 us into the block).
  - Per block, out-proj results accumulate into a [128, 4096] SBUF tile
    written out with four 256 KB DMAs; tail copies alternate ScalarE/DVE.
  - Host sums the 8 per-core partials (the TP all-reduce) and adds
    (bv @ Wo + bo), which commutes with attention since softmax rows sum
    to 1.

  Queue-discipline invariant (hardware deadlocks otherwise, CoreSim does not
  model it): the in-order sync queue carries gated input-chunk triggers, and
  every slot those triggers wait on is freed by work emitted before the first
  norm shuffle DMA that shares the queue.
"""
import sys
from contextlib import ExitStack

import numpy as np

sys.path.insert(0, "/opt/trn_rl_repo")

import ml_dtypes  # noqa: E402
import concourse.bass as bass  # noqa: E402
import concourse.mybir as mybir  # noqa: E402
import concourse.tile as tile  # noqa: E402
from concourse import bacc  # noqa: E402
from concourse.bass_utils import run_bass_kernel_spmd  # noqa: E402

BF16 = mybir.dt.bfloat16
F32 = mybir.dt.float32
AF = mybir.ActivationFunctionType

NCORES = 8
B, L, E, H = 2, 2048, 1024, 16
S = L
D = E // H            # 64 head dim
R = B * L             # 4096 total rows
HC = H // NCORES      # 2 heads per core
EC = HC * D           # 128 channel slice per core
KT = E // 128         # 8 contraction tiles
NT = L // 512         # 4 512-wide row tiles per batch
ST = S // 128         # 16 key tiles per batch
STN = ST // NT        # 4 key tiles per 512-row block
DP1 = D + 1           # 65: head dim + ones column
NBLK = B * NT         # 8 query blocks overall

# batch 0 projects as a dedicated phase (it is DMA-bound; streaming it into
# attention was measured slower: the exp stream then runs at DMA pace).
# Batch-1 work streams into attention steps under need-by guards.
B0_PHASE = [("k", 0), ("v", 0), ("k", 1), ("v", 1), ("k", 2), ("v", 2),
            ("k", 3), ("v", 3), ("q", 0), ("q", 1), ("q", 2), ("q", 3)]
B0_FEED = []
B1_FEED = [("k", 0), ("v", 0), ("k", 1), ("v", 1), ("k", 2), ("v", 2),
           ("q", 0), ("k", 3), ("v", 3), ("q", 1), ("q", 2), ("q", 3)]


def build_nc():
    nc = bacc.Bacc("TRN2", target_bir_lowering=False, num_devices=NCORES)

    qT = nc.declare_dram_parameter("qT", [NBLK, 128, KT, 512], BF16, isOutput=False)
    kT = nc.declare_dram_parameter("kT", [NBLK, 128, KT, 512], BF16, isOutput=False)
    vT = nc.declare_dram_parameter("vT", [NBLK, 128, KT, 512], BF16, isOutput=False)
    wq = nc.declare_dram_parameter("wq", [128, KT * EC], BF16, isOutput=False)
    wk = nc.declare_dram_parameter("wk", [128, KT * EC], BF16, isOutput=False)
    wv = nc.declare_dram_parameter("wv", [128, KT * EC], BF16, isOutput=False)
    wo = nc.declare_dram_parameter("wo", [128, E], BF16, isOutput=False)
    bq = nc.declare_dram_parameter("bq", [EC, 1], F32, isOutput=False)
    bk = nc.declare_dram_parameter("bk", [EC, 1], F32, isOutput=False)
    outO = nc.declare_dram_parameter("outO", [NBLK, 128, KT * 512], BF16,
                                     isOutput=True)

    with tile.TileContext(nc) as tc, ExitStack() as ctx:
        consts = ctx.enter_context(tc.tile_pool(name="consts", bufs=1))
        # 9 bufs: every batch-1 chunk trigger on the in-order sync queue then
        # waits only on batch-0/early-feed consumption, which is all emitted
        # before the first norm shuffle DMA — the sync queue provably drains
        # ahead of the attention-phase norm chain (no cross-queue deadlock)
        xs_pool = ctx.enter_context(tc.tile_pool(name="xs", bufs=9))
        exp_pool = ctx.enter_context(tc.tile_pool(name="expp", bufs=4))
        ot_pool = ctx.enter_context(tc.tile_pool(name="otp", bufs=3))
        pou_pool = ctx.enter_context(tc.tile_pool(name="poup", bufs=6))
        rc_pool = ctx.enter_context(tc.tile_pool(name="rcp", bufs=4))
        obt_pool = ctx.enter_context(tc.tile_pool(name="obtp", bufs=2))
        # PSUM banks: sc 2x[128,1024] (4) + pv 2x[128,512] (2) + pp 2x[128,512] (2)
        psum_sc = ctx.enter_context(tc.tile_pool(name="psc", bufs=2, space="PSUM"))
        psum_pv = ctx.enter_context(tc.tile_pool(name="ppv", bufs=2, space="PSUM"))
        psum_pp = ctx.enter_context(tc.tile_pool(name="ppp", bufs=2, space="PSUM"))

        # ---- weights staging (host pre-arranged, contiguous); wk first since
        # the k projections consume it first.
        wq_sb = consts.tile([128, KT, EC], BF16, tag="wq")
        wk_sb = consts.tile([128, KT, EC], BF16, tag="wk")
        wv_sb = consts.tile([128, KT, EC], BF16, tag="wv")
        wo_sb = consts.tile([128, KT, EC], BF16, tag="wo")
        nc.sync.dma_start(wk_sb[:], wk[:].rearrange("p (ko m) -> p ko m", m=EC))
        bq_sb = consts.tile([EC, 1], F32, tag="bq")
        bk_sb = consts.tile([EC, 1], F32, tag="bk")
        nc.gpsimd.dma_start(bq_sb[:], bq[:])
        nc.gpsimd.dma_start(bk_sb[:], bk[:])

        # per-(batch, n-tile) activation tiles
        qpT = [[consts.tile([128, 512], BF16, tag=f"qpT{b}_{n}", name=f"qpT{b}_{n}")
                for n in range(NT)] for b in range(B)]
        kpT = [[consts.tile([128, 512], BF16, tag=f"kpT{b}_{n}", name=f"kpT{b}_{n}")
                for n in range(NT)] for b in range(B)]
        vp = [[consts.tile([128, STN, 2 * DP1], BF16, tag=f"vp{b}_{n}",
                           name=f"vp{b}_{n}")
               for n in range(NT)] for b in range(B)]
        for b in range(B):
            for n in range(NT):
                nc.vector.memset(vp[b][n][:, :, D], 1.0)
                nc.vector.memset(vp[b][n][:, :, 2 * D + 1], 1.0)

        # input chunk DMAs, emitted in exact consumption order on the sync
        # queue; the xs pool (6 bufs) gates prefetch depth.  Remaining weight
        # DMAs are interleaved right where they are first needed.
        XSRC = {"k": kT, "v": vT, "q": qT}
        staged = {}
        for b, order in ((0, B0_PHASE + B0_FEED), (1, B1_FEED)):
            for i, (name, n) in enumerate(order):
                xt = xs_pool.tile([128, KT, 512], BF16, tag="xs",
                                  name=f"xt{name}{b}_{n}")
                nc.sync.dma_start(xt[:], XSRC[name][b * NT + n])
                staged[(b, name, n)] = xt
                if b == 0 and i == 0:
                    nc.sync.dma_start(
                        wv_sb[:], wv[:].rearrange("p (ko m) -> p ko m", m=EC))
                if b == 0 and i == 1:
                    nc.sync.dma_start(
                        wq_sb[:], wq[:].rearrange("p (ko m) -> p ko m", m=EC))
                    nc.sync.dma_start(
                        wo_sb[:], wo[:].rearrange("p (m o) -> p m o", o=EC))

        # deferred work queues
        pending = []       # (pou0, pou1, obt, blk, w, off) awaiting norm
        pending_proj = []  # (ot, obt, blk, m, w, off) out-projection tiles
        pe_feed = []       # (key, closure) batch-1 projection work items
        feed_done = set()  # keys of completed feed items
        obt_live = {}      # blk -> [obt tile, columns written]

        def norm_pending():
            while pending:
                pou0, pou1, obt, blk, w, off = pending.pop(0)
                pp_ = w // 4   # shuffle partitions (16 B per partition)
                wc = 4         # reciprocal columns per head
                # free-size-bound DVE: shuffle the [1,w] sum rows into a
                # [128,*] layout via DMA so one reciprocal covers both heads
                # in a few columns; the permutation cancels on unshuffle.
                smT = rc_pool.tile([128, 8], F32, tag="smT")
                nc.sync.dma_start(smT[0:pp_, 0:wc], pou0[D:DP1, 0:w])
                nc.sync.dma_start(smT[0:pp_, wc:2 * wc], pou1[D:DP1, 0:w])
                smR = rc_pool.tile([128, 8], F32, tag="smR")
                nc.vector.reciprocal(smR[0:pp_, 0:2 * wc], smT[0:pp_, 0:2 * wc])
                ot = ot_pool.tile([128, 512], BF16, tag="ot")
                for h, pou in ((0, pou0), (1, pou1)):
                    rcp = rc_pool.tile([1, 512], F32, tag="rcp")
                    nc.sync.dma_start(rcp[:, 0:w],
                                      smR[0:pp_, h * wc:(h + 1) * wc])
                    rcb = rc_pool.tile([D, 512], F32, tag="rcb")
                    nc.gpsimd.partition_broadcast(rcb[:, 0:w], rcp[:, 0:w])
                    nc.vector.tensor_mul(
                        ot[h * D:(h + 1) * D, 0:w], pou[0:D, 0:w], rcb[:, 0:w]
                    )
                for m in range(KT):
                    pending_proj.append((ot, obt, blk, m, w, off))

        def proj_one(tail=False):
            # one 128 x w partial out-projection tile; in the tail the copies
            # alternate DVE/ScalarE (idle then, Copy shares the exp table) so
            # the final 8 tiles pipeline two-wide
            ot, obt, blk, m, w, off = pending_proj.pop(0)
            pt = psum_pp.tile([128, 512], F32, tag="pp")
            nc.tensor.matmul(
                pt[:, 0:w], lhsT=wo_sb[:, m, :], rhs=ot[:, 0:w],
                start=True, stop=True,
            )
            if tail and m % 2 == 1:
                nc.scalar.activation(
                    obt[:, m * 512 + off:m * 512 + off + w], pt[:, 0:w],
                    AF.Copy)
            else:
                nc.vector.tensor_copy(
                    obt[:, m * 512 + off:m * 512 + off + w], pt[:, 0:w])
            obt_live[blk][1] += w
            done = obt_live[blk][1]
            quarter = KT * 512 // 4
            if done % quarter == 0:
                qi = done // quarter - 1
                nc.gpsimd.dma_start(
                    outO[blk][:, qi * quarter:(qi + 1) * quarter],
                    obt[:, qi * quarter:(qi + 1) * quarter])

        def drain_one():
            if pending_proj:
                proj_one()

        def feed_one():
            if pe_feed:
                key, it = pe_feed.pop(0)
                it()
                if key is not None:
                    feed_done.add(key)

        def feed_until(key):
            while key not in feed_done:
                k2, it = pe_feed.pop(0)
                it()
                if k2 is not None:
                    feed_done.add(k2)

        def flush_all():
            norm_pending()
            while pending_proj:
                proj_one(tail=True)

        def proj_items(b, name, n, use_scalar):
            """Closures emitting the projection of chunk (b, name, n), each
            bounded to ~1 us of PE work so they slot into attention steps.
            The last closure of a chunk completes key (name, n)."""
            xt = staged.pop((b, name, n))
            w_sb = {"k": wk_sb, "v": wv_sb, "q": wq_sb}[name]
            if name == "v":
                # natural-layout vp via direct matmuls: output partitions are
                # keys, so no PE transpose and no extra PSUM tag needed
                items = []
                for sblk in range(STN):
                    def it_v(sblk=sblk, xt=xt, b=b, n=n):
                        ps = psum_pp.tile([128, 128], F32, tag="pp",
                                          name="psv")
                        for kt in range(KT):
                            nc.tensor.matmul(
                                ps[:],
                                lhsT=xt[:, kt, sblk * 128:(sblk + 1) * 128],
                                rhs=w_sb[:, kt, :],
                                start=(kt == 0),
                                stop=(kt == KT - 1),
                            )
                        nc.vector.tensor_copy(
                            vp[b][n][:, sblk, 0:D], ps[:, 0:D])
                        nc.vector.tensor_copy(
                            vp[b][n][:, sblk, DP1:DP1 + D], ps[:, D:2 * D])
                    items.append(it_v)
                return items
            dest = (kpT if name == "k" else qpT)[b][n]
            bias = bk_sb if name == "k" else bq_sb
            state = {}

            def it1():
                ps = psum_pp.tile([128, 512], F32, tag="pp", name="pskq")
                state["ps"] = ps
                for kt in range(KT // 2):
                    nc.tensor.matmul(
                        ps[:], lhsT=w_sb[:, kt, :], rhs=xt[:, kt, :],
                        start=(kt == 0), stop=False,
                    )

            def it2():
                ps = state["ps"]
                for kt in range(KT // 2, KT):
                    nc.tensor.matmul(
                        ps[:], lhsT=w_sb[:, kt, :], rhs=xt[:, kt, :],
                        start=False, stop=(kt == KT - 1),
                    )
                if use_scalar:
                    nc.scalar.activation(dest[:], ps[:], AF.Identity,
                                         bias=bias[:])
                else:
                    nc.vector.tensor_tensor(
                        dest[:], ps[:], bias[:].to_broadcast((EC, 512)),
                        mybir.AluOpType.add,
                    )
            return [it1, it2]

        def project_phase():
            """Tiny startup phase: just the chunks attn(0,0) step 0 needs."""
            for name, n in B0_PHASE:
                for it in proj_items(0, name, n, use_scalar=True):
                    it()
                feed_done.add((0, name, n))

        def attention(b, lt, w=512, off=0):
            """One w-query block: both heads, full softmax + PV.

            Scores are computed one step ahead of exp so feed/drain work on
            the in-order PE queue does not pause the exp stream. Feed items
            (batch-1 projections) run at early steps, out-projection drains
            at late steps (their ot input is ready only ~7 us into the
            block after the deferred norm chain).
            """
            blk = b * NT + lt
            if off == 0:
                obt = obt_pool.tile([128, KT * 512], BF16, tag="obt",
                                    name=f"obt{blk}")
                obt_live[blk] = [obt, 0, w != 512]
            obt = obt_live[blk][0]
            po = []
            for h in range(HC):
                p = psum_pv.tile([128, 512], F32, tag="pv", name=f"po{h}")
                po.append(p)

            def scores(st):
                feed_until((b, "k", st // STN))
                feed_until((b, "v", st // STN))
                if st == 0:
                    feed_until((b, "q", lt))
                ps = psum_sc.tile([128, 1024], F32, tag="sc", name="sc")
                for h in range(HC):
                    nc.tensor.matmul(
                        ps[:, h * w:(h + 1) * w],
                        lhsT=kpT[b][st // STN][h * D:(h + 1) * D,
                                               (st % STN) * 128:(st % STN + 1) * 128],
                        rhs=qpT[b][lt][h * D:(h + 1) * D, off:off + w],
                        start=True,
                        stop=True,
                        tile_position=(h * D, 0),
                    )
                return ps

            ps = scores(0)
            for st in range(ST):
                ex = exp_pool.tile([128, 1024], BF16, tag="exp")
                nc.scalar.activation(ex[:, 0:2 * w], ps[:, 0:2 * w], AF.Exp)
                if st + 1 < ST:
                    ps = scores(st + 1)
                for h in range(HC):
                    nc.tensor.matmul(
                        po[h][0:DP1, 0:w],
                        lhsT=vp[b][st // STN][:, st % STN, h * DP1:(h + 1) * DP1],
                        rhs=ex[:, h * w:(h + 1) * w],
                        start=(st == 0),
                        stop=(st == ST - 1),
                    )
                if st == 1:
                    norm_pending()
                if st >= ST - KT - 1 and pending_proj:
                    drain_one()
                elif st < ST - 3 and not (b == 0 and lt <= 1 and st % 2 == 0):
                    # paced on early blocks; never in the last steps (they
                    # collide with the block-end po handoff)
                    feed_one()
            # free the po PSUM banks promptly; norm works off the SBUF copy
            pou0 = pou_pool.tile([DP1, 512], F32, tag="pou", name="pou0")
            pou1 = pou_pool.tile([DP1, 512], F32, tag="pou", name="pou1")
            nc.vector.tensor_copy(pou0[:, 0:w], po[0][0:DP1, 0:w])
            nc.vector.tensor_copy(pou1[:, 0:w], po[1][0:DP1, 0:w])
            pending.append((pou0, pou1, obt, blk, w, off))

        project_phase()
        for b, order in ((0, B0_FEED), (1, B1_FEED)):
            for name, n in order:
                items = proj_items(b, name, n, use_scalar=False)
                for it in items[:-1]:
                    pe_feed.append((None, it))
                pe_feed.append(((b, name, n), items[-1]))
        for lt in range(NT):
            attention(0, lt)
        for lt in range(NT):
            attention(1, lt)
        flush_all()

    nc.compile()
    return nc


_NC_CACHE = {}


def _get_nc():
    if "nc" not in _NC_CACHE:
        _NC_CACHE["nc"] = build_nc()
    return _NC_CACHE["nc"]


def _prearrange(w):
    # [E, EC] -> [128, KT*EC] partition-major so the device DMA is contiguous
    bf = ml_dtypes.bfloat16
    return np.ascontiguousarray(
        w.reshape(KT, 128, EC).transpose(1, 0, 2).reshape(128, KT * EC)
    ).astype(bf)


def kernel(q, k, v, Wq, bq, Wk, bk, Wv, bv, Wo, bo, _trace=False, _tmpdir=None):
    bf = ml_dtypes.bfloat16
    scale = np.float32(1.0 / np.sqrt(D))  # 0.125, exact

    def _stage_x(x):
        # [B, L, E] -> [NBLK, 128, KT, 512] chunk-contiguous staging layout
        xt = np.asarray(x, np.float32).reshape(B, NT, 512, KT, 128)
        return np.ascontiguousarray(
            xt.transpose(0, 1, 4, 3, 2).reshape(NBLK, 128, KT, 512)
        ).astype(bf)

    qTh = _stage_x(q)
    kTh = _stage_x(k)
    vTh = _stage_x(v)
    Wq = np.asarray(Wq, np.float32)
    Wk = np.asarray(Wk, np.float32)
    Wv = np.asarray(Wv, np.float32)
    Wo = np.asarray(Wo, np.float32)

    in_maps = []
    for c in range(NCORES):
        sl = slice(c * EC, (c + 1) * EC)
        in_maps.append({
            "qT": qTh,
            "kT": kTh,
            "vT": vTh,
            "wq": _prearrange(Wq[:, sl] * scale),
            "wk": _prearrange(Wk[:, sl]),
            "wv": _prearrange(Wv[:, sl]),
            "wo": np.ascontiguousarray(Wo[sl, :]).astype(bf),
            "bq": (np.asarray(bq, np.float32)[sl] * scale).reshape(EC, 1).copy(),
            "bk": np.asarray(bk, np.float32)[sl].reshape(EC, 1).copy(),
        })

    nc = _get_nc()
    res = run_bass_kernel_spmd(
        nc, in_maps, list(range(NCORES)), trace=_trace, tmpdir=_tmpdir
    )
    # sum the per-core partial outputs (the all-reduce of the TP sharding)
    acc = np.zeros((E, R), np.float32)
    for c in range(NCORES):
        # [NBLK, 128, KT*512] -> [E, R]
        part = np.asarray(res.results[c]["outO"], np.float32)
        acc += part.reshape(NBLK, 128, KT, 512).transpose(2, 1, 0, 3).reshape(E, R)
    out = np.ascontiguousarray(acc.T)  # [R, E]
    # bv passes through attention unchanged (softmax rows sum to 1):
    # out += bv @ Wo + bo
    host_bias = (
        np.asarray(bv, np.float64) @ np.asarray(Wo, np.float64)
        + np.asarray(bo, np.float64)
    ).astype(np.float32)
    out += host_bias[None, :]
    if _trace:
        return out.reshape(B, L, E), res
    return out.reshape(B, L, E)
